# revision 1
# baseline (speedup 1.0000x reference)
"""Trainium2 Bass kernel for nn_BuildModel_3796751089795 (ON-LSTM-style RNN).

Model per reference:
  - sequential ON-LSTM cell over T=128 steps (cumax master gates L=3, CH=128)
  - per-step sliding-window (K=10) "theme"/"conv" head
  - output projection + per-batch-row selection at t = v_lengths[b]-1

Key algorithmic facts exploited:
  * only cur_output[b] = rnn[t_b, b] @ out_w + out_b is needed (t_b =
    v_lengths[b]-1), so the windowed conv/theme head is computed ONLY at t_b
    per batch row (gathered from the stored h/dist sequence), not at all T.
  * the x_t @ kernel_w part of the gate preactivation is computed inside the
    step loop by accumulating into the same PSUM tile as the h @ rec_w part
    (2 F-chunk matmuls against an on-chip transposed copy of X).
  * sigmoid(x) = 0.5*tanh(0.5x)+0.5 so every gate nonlinearity uses the one
    ACT table set that also contains exp (no per-step table switches).
  * local_dis softmax is computed on the gathered 10-tap dist window only.

Sharding: data-parallel over batch, B=512 -> 8 cores x 64 rows.

Self-contained: hardcodes all shapes; no file reads.
"""

import numpy as np
import ml_dtypes

import concourse.bass as bass
import concourse.tile as tile
from concourse import mybir
from concourse.bass_utils import run_bass_kernel_spmd
from concourse.masks import make_identity

F32 = mybir.dt.float32
F32R = mybir.dt.float32r
BF16 = mybir.dt.bfloat16
I32 = mybir.dt.int32
AF = mybir.ActivationFunctionType
OP = mybir.AluOpType
AX = mybir.AxisListType

B, T, F, H, L, K, LAB = 512, 128, 256, 384, 3, 10, 25
CH = H // L            # 128
GATES = 4 * H + 2 * L  # 1542
NCORES = 8
BL = B // NCORES       # 64 batch rows per core
PAD = K - 1            # 9 zero timesteps of h-prefix for window taps t<0

# reordered gate-column layout (see _prep_shared):
#   cols [0:1536): 3 blocks of 512 = [f_l(128) i_l(128) o_l(128) ci_l(128)]
#   cols [1536:1542): fm(3) im(3) preactivations
NG = 1536


def _gate_perm_scale():
    """Return (perm, scale): new_W[:, j] = old_W[:, perm[j]] * scale[j]."""
    perm = np.zeros(GATES, np.int64)
    scale = np.ones(GATES, np.float32)
    for l in range(L):
        base = l * 4 * CH
        for gi in range(4):   # f, i, o, ci
            perm[base + gi * CH: base + (gi + 1) * CH] = np.arange(
                2 * L + gi * H + l * CH, 2 * L + gi * H + (l + 1) * CH)
        # f/i/o go through sigmoid-via-tanh: pre-scale by 0.5; ci stays
        scale[base: base + 3 * CH] = 0.5
    perm[NG:] = np.arange(2 * L)
    return perm, scale


def _prep_shared(kernel_w, kernel_b, rec_w, rec_b, scale_w, scale_b,
                 rescale_w, rescale_b, conv_w, conv_b, out_w, out_b):
    """Host-side weight preprocessing (replicated across cores)."""
    perm, colscale = _gate_perm_scale()

    def reorder(v):  # v [..., GATES]
        return (v[..., perm] * colscale).astype(np.float32)

    wpre = reorder(kernel_w[:F])                     # [256, 1542]
    wrec = reorder(rec_w[:H])                        # [384, 1542]
    cb = reorder(kernel_b + rec_b + kernel_w[F] + rec_w[H])[None, :]   # [1,1542]
    trow = reorder(kernel_w[F] + rec_w[H])           # [1542]
    trow_rep = np.tile(trow[None, :], (BL, 1)).astype(np.float32)      # [64,1542]

    # conv_w [O=384, Hin=384, K=10] -> convT [128(h'), (k,hc,oc,o)] bf16
    convT = np.zeros((CH, K * 3 * 3 * CH), np.float32)
    for k in range(K):
        for hc in range(3):
            for oc in range(3):
                blk = conv_w[oc * CH:(oc + 1) * CH, hc * CH:(hc + 1) * CH, k].T
                off = ((k * 3 + hc) * 3 + oc) * CH
                convT[:, off:off + CH] = blk
    convT = convT.astype(ml_dtypes.bfloat16)

    scaleW = np.zeros((CH, 3 * 64), np.float32)      # lhsT chunks [h',feat]
    for hc in range(3):
        scaleW[:, hc * 64:(hc + 1) * 64] = scale_w[hc * CH:(hc + 1) * CH, :] / 10.0
    scalebT = scale_b.reshape(64, 1).astype(np.float32)
    rescaleW = rescale_w.astype(np.float32)          # [64, 384] = [K, (oc,o)]
    rescbT = np.zeros((CH, 3), np.float32)
    convbT = np.zeros((CH, 3), np.float32)
    outW = np.zeros((CH, 3 * LAB), np.float32)
    for oc in range(3):
        rescbT[:, oc] = 0.5 * rescale_b[oc * CH:(oc + 1) * CH]
        convbT[:, oc] = conv_b[oc * CH:(oc + 1) * CH]
        outW[:, oc * LAB:(oc + 1) * LAB] = out_w[oc * CH:(oc + 1) * CH, :]
    outb_rep = np.tile(out_b[None, :], (BL, 1)).astype(np.float32)

    return dict(
        wpre0=np.ascontiguousarray(wpre[:128]), wpre1=np.ascontiguousarray(wpre[128:]),
        wrec0=np.ascontiguousarray(wrec[:128]), wrec1=np.ascontiguousarray(wrec[128:256]),
        wrec2=np.ascontiguousarray(wrec[256:]),
        cbw=cb.astype(np.float32), trow_rep=trow_rep,
        convT=convT, scaleW=scaleW, scalebT=scalebT,
        rescaleW=rescaleW, rescbT=rescbT, convbT=convbT,
        outW=outW, outb_rep=outb_rep,
        ones1=np.ones((1, BL), np.float32),
    )


def build_nc(t_steps=T, debug_hseq=False):
    """Build the Bass module for one core (B-shard of 64 rows)."""
    nc = bass.Bass()
    ROWS = BL * t_steps
    HS_ROWS = (t_steps + PAD) * BL

    # ---------------- I/O ----------------
    d_x = nc.dram_tensor("x", [BL, t_steps, F], F32, kind="ExternalInput")
    d_wpre = [nc.dram_tensor(f"wpre{i}", [128, GATES], F32R, kind="ExternalInput")
              for i in range(2)]
    d_wrec = [nc.dram_tensor(f"wrec{i}", [128, GATES], F32R, kind="ExternalInput")
              for i in range(3)]
    d_cbw = nc.dram_tensor("cbw", [1, GATES], F32R, kind="ExternalInput")
    d_trow = nc.dram_tensor("trow_rep", [BL, GATES], F32, kind="ExternalInput")
    d_convT = nc.dram_tensor("convT", [CH, K * 9 * CH], BF16, kind="ExternalInput")
    d_scaleW = nc.dram_tensor("scaleW", [CH, 192], F32, kind="ExternalInput")
    d_scaleb = nc.dram_tensor("scalebT", [64, 1], F32, kind="ExternalInput")
    d_rescaleW = nc.dram_tensor("rescaleW", [64, H], F32, kind="ExternalInput")
    d_rescb = nc.dram_tensor("rescbT", [CH, 3], F32, kind="ExternalInput")
    d_convb = nc.dram_tensor("convbT", [CH, 3], F32, kind="ExternalInput")
    d_outW = nc.dram_tensor("outW", [CH, 3 * LAB], F32, kind="ExternalInput")
    d_outb = nc.dram_tensor("outb_rep", [BL, LAB], F32, kind="ExternalInput")
    d_ones1 = nc.dram_tensor("ones1", [1, BL], F32R, kind="ExternalInput")
    d_gidx = nc.dram_tensor("gidx", [128, 5], I32, kind="ExternalInput")

    hseq_kind = "ExternalOutput" if debug_hseq else "Internal"
    d_hseq = nc.dram_tensor("hseq", [HS_ROWS, H + 1], F32, kind=hseq_kind)
    d_out = nc.dram_tensor("cur_out", [BL, LAB], F32, kind="ExternalOutput")
    d_dscr = nc.dram_tensor("dscr", [1, K * BL], F32)
    d_dbg_g = d_dbg_sm = None
    if debug_hseq:
        d_dbg_g = nc.dram_tensor("dbg_g", [t_steps * BL, NG], F32, kind="ExternalOutput")
        d_dbg_sm = nc.dram_tensor("dbg_sm", [t_steps * BL, 32], F32, kind="ExternalOutput")

    with tile.TileContext(nc) as tc:
        with (
            tc.tile_pool(name="singles", bufs=1) as singles,
            tc.tile_pool(name="post", bufs=1) as post_p,
        ):
            # ------------- load constants -------------
            ident = singles.tile([128, 128], F32)
            make_identity(nc, ident[:])
            id64 = ident[0:64, 0:64]
            wpre_sb = [singles.tile([128, GATES], F32R, tag=f"wpre{i}", name=f"wpre{i}")
                       for i in range(2)]
            wrec_sb = [singles.tile([128, GATES], F32R, tag=f"wrec{i}", name=f"wrec{i}")
                       for i in range(3)]
            for i in range(2):
                nc.sync.dma_start(wpre_sb[i][:], d_wpre[i][:])
            for i in range(3):
                nc.sync.dma_start(wrec_sb[i][:], d_wrec[i][:])
            cbw_sb = singles.tile([1, GATES], F32R)
            nc.sync.dma_start(cbw_sb[:], d_cbw[:])
            trow_sb = singles.tile([BL, GATES], F32)
            nc.sync.dma_start(trow_sb[:], d_trow[:])
            convT_sb = singles.tile([CH, K * 9 * CH], BF16)
            nc.sync.dma_start(convT_sb[:], d_convT[:])
            scaleW_sb = singles.tile([CH, 192], F32)
            nc.sync.dma_start(scaleW_sb[:], d_scaleW[:])
            scaleb_sb = singles.tile([64, 1], F32)
            nc.sync.dma_start(scaleb_sb[:], d_scaleb[:])
            rescaleW_sb = singles.tile([64, H], F32)
            nc.sync.dma_start(rescaleW_sb[:], d_rescaleW[:])
            rescb_sb = singles.tile([CH, 3], F32)
            nc.sync.dma_start(rescb_sb[:], d_rescb[:])
            convb_sb = singles.tile([CH, 3], F32)
            nc.sync.dma_start(convb_sb[:], d_convb[:])
            outW_sb = singles.tile([CH, 3 * LAB], F32)
            nc.sync.dma_start(outW_sb[:], d_outW[:])
            outb_sb = singles.tile([BL, LAB], F32)
            nc.sync.dma_start(outb_sb[:], d_outb[:])
            ones1_sb = singles.tile([1, BL], F32R)
            nc.sync.dma_start(ones1_sb[:], d_ones1[:])
            gidx_sb = singles.tile([128, 5], I32)
            nc.sync.dma_start(gidx_sb[:], d_gidx[:])

            # zero the h-prefix rows of hseq
            zrow = singles.tile([128, H + 1], F32)
            nc.vector.memset(zrow[:], 0.0)
            zr = PAD * BL  # 576
            for r0 in range(0, zr, 128):
                n = min(128, zr - r0)
                nc.sync.dma_start(d_hseq[r0:r0 + n, :], zrow[:n, :])

            # ------------- phase 1: X -> XT (F-major) -------------
            # XT[fc][f, t*64+b] = X[b, t, f]
            xt_sb = [singles.tile([128, ROWS], F32R, tag=f"xt{i}", name=f"xt{i}")
                     for i in range(2)]
            x_tmaj = d_x[:].rearrange("b t f -> t b f")
            with (
                tc.tile_pool(name="xrow", bufs=4) as xrow_p,
                tc.tile_pool(name="trps", bufs=4, space="PSUM") as trps_p,
            ):
                for rt in range(ROWS // 128):
                    xr = xrow_p.tile([128, F], F32, tag="xrow", name="xr")
                    t0 = rt * 2
                    nc.sync.dma_start(xr[0:64, :], x_tmaj[t0, :, :])
                    nc.sync.dma_start(xr[64:128, :], x_tmaj[t0 + 1, :, :])
                    for fc in range(2):
                        pt = trps_p.tile([128, 128], F32, tag="xtp", name="pt")
                        nc.tensor.transpose(pt[:], xr[:, fc * 128:(fc + 1) * 128],
                                            ident[:])
                        nc.scalar.copy(xt_sb[fc][:, rt * 128:(rt + 1) * 128], pt[:])

            # ------------- phase 2: recurrence -------------
            with (
                tc.tile_pool(name="xo", bufs=2, space="PSUM") as xo_p,
                tc.tile_pool(name="smallps", bufs=1, space="PSUM") as smallps_p,
                tc.tile_pool(name="gates", bufs=2) as gates_p,
                tc.tile_pool(name="state", bufs=2) as state_p,
                tc.tile_pool(name="sm", bufs=3) as sm_p,
            ):
                fmim_ring = smallps_p.tile([BL, 512], F32, tag="fmim", name="fmim")
                tr_ring = smallps_p.tile([128, 384], F32, tag="trring", name="trring")

                hT_prev = None   # [3] tiles [128, 64] = h.T chunks
                c_prev = None    # [64, 384]

                for t in range(t_steps):
                    # ---- gate preactivation matmuls (fp32r views) ----
                    xo = xo_p.tile([BL, NG], F32, tag="xo", name="xo")
                    slot = fmim_ring[:, (t % 64) * 8:(t % 64) * 8 + 6]
                    for nt in range(3):
                        cs = slice(nt * 512, (nt + 1) * 512)
                        nc.tensor.matmul(
                            xo[:, cs], ones1_sb[:],
                            cbw_sb[:, cs], start=True, stop=False)
                        for fc in range(2):
                            nc.tensor.matmul(
                                xo[:, cs],
                                xt_sb[fc][:, t * BL:(t + 1) * BL],
                                wpre_sb[fc][:, cs],
                                start=False, stop=(t == 0 and fc == 1))
                        if t > 0:
                            for hc in range(3):
                                nc.tensor.matmul(
                                    xo[:, cs], hT_prev[hc][:],
                                    wrec_sb[hc][:, cs],
                                    start=False, stop=(hc == 2))
                    nc.tensor.matmul(slot, ones1_sb[:],
                                     cbw_sb[:, NG:],
                                     start=True, stop=False)
                    for fc in range(2):
                        nc.tensor.matmul(
                            slot, xt_sb[fc][:, t * BL:(t + 1) * BL],
                            wpre_sb[fc][:, NG:],
                            start=False, stop=(t == 0 and fc == 1))
                    if t > 0:
                        for hc in range(3):
                            nc.tensor.matmul(
                                slot, hT_prev[hc][:],
                                wrec_sb[hc][:, NG:],
                                start=False, stop=(hc == 2))
                    else:
                        # t=0: Tint=0 -> remove the trow part of the cb bias row
                        for nt in range(3):
                            cs = slice(nt * 512, (nt + 1) * 512)
                            nc.vector.tensor_tensor(
                                xo[:, cs], xo[:, cs], trow_sb[:, cs], op=OP.subtract)
                        nc.vector.tensor_tensor(
                            slot, slot, trow_sb[:, NG:], op=OP.subtract)

                    # ---- fm/im softmax+cumsum chain (tiny) ----
                    # sm cols: 0:6 e (1,4 become u01,u45), 8:10 sums, 10:12 recips,
                    # 12:18 [fm0 fm1 1 | 1 im1 im2], 18:27 s1 s2 s3, 27 fm0+fm1
                    sm = sm_p.tile([BL, 32], F32, tag="sm", name="sm")
                    nc.scalar.activation(sm[:, 0:6], slot, AF.Exp)
                    nc.vector.tensor_reduce(
                        sm[:, 8:10], sm[:, 0:6].rearrange("p (a b) -> p a b", b=3),
                        axis=AX.X, op=OP.add)
                    nc.vector.tensor_tensor(sm[:, 1:2], sm[:, 0:1], sm[:, 1:2],
                                            op=OP.add)   # u01 = e0+e1
                    nc.vector.tensor_tensor(sm[:, 4:5], sm[:, 5:6], sm[:, 4:5],
                                            op=OP.add)   # u45 = e4+e5
                    nc.vector.reciprocal(sm[:, 10:12], sm[:, 8:10])
                    nc.vector.memset(sm[:, 14:16], 1.0)  # fm2 = im0 = 1
                    nc.vector.tensor_scalar(sm[:, 12:14], sm[:, 0:2],
                                            scalar1=sm[:, 10:11], scalar2=None,
                                            op0=OP.mult)  # fm0, fm1
                    nc.vector.tensor_scalar(sm[:, 16:18], sm[:, 4:6],
                                            scalar1=sm[:, 11:12], scalar2=None,
                                            op0=OP.mult)  # im1, im2
                    nc.vector.tensor_tensor(sm[:, 18:21], sm[:, 12:15], sm[:, 15:18],
                                            op=OP.mult)      # s1 = fm*im
                    nc.vector.tensor_tensor(sm[:, 21:24], sm[:, 12:15], sm[:, 18:21],
                                            op=OP.subtract)  # s2 = fm-s1
                    nc.vector.tensor_tensor(sm[:, 24:27], sm[:, 15:18], sm[:, 18:21],
                                            op=OP.subtract)  # s3 = im-s1

                    # ---- gate nonlinearities ----
                    gates = gates_p.tile([BL, NG], F32, tag="gates", name="gates")
                    nc.scalar.activation(gates[:, 0:512], xo[:, 0:512], AF.Tanh)
                    nc.scalar.activation(gates[:, 512:1536], xo[:, 512:1536], AF.Tanh)
                    for l in range(3):
                        nc.gpsimd.tensor_scalar(
                            gates[:, l * 512:l * 512 + 384],
                            gates[:, l * 512:l * 512 + 384],
                            scalar1=0.5, scalar2=0.5, op0=OP.mult, op1=OP.add)

                    # ---- cell update ----
                    # F_l = s1*f + s2 ; I_l = s1*i + s3 ; c = c_prev*F + ci*I
                    Iv = sm_p.tile([BL, H], F32, tag="Iv", name="Iv")
                    for l in range(3):
                        nc.gpsimd.tensor_scalar(
                            Iv[:, l * 128:(l + 1) * 128],
                            gates[:, l * 512 + 128:l * 512 + 256],
                            scalar1=sm[:, 18 + l:19 + l], scalar2=sm[:, 24 + l:25 + l],
                            op0=OP.mult, op1=OP.add)
                    c_new = state_p.tile([BL, H], F32, tag="c", name="c_new")
                    ci_view = gates[:].rearrange("p (l c) -> p l c", c=512)[:, :, 384:512]
                    nc.vector.tensor_tensor(
                        c_new[:].rearrange("p (l c) -> p l c", c=128), ci_view,
                        Iv[:].rearrange("p (l c) -> p l c", c=128), op=OP.mult)
                    if t > 0:
                        Fv = sm_p.tile([BL, H], F32, tag="Fv", name="Fv")
                        for l in range(3):
                            nc.vector.tensor_scalar(
                                Fv[:, l * 128:(l + 1) * 128],
                                gates[:, l * 512:l * 512 + 128],
                                scalar1=sm[:, 18 + l:19 + l],
                                scalar2=sm[:, 21 + l:22 + l],
                                op0=OP.mult, op1=OP.add)
                        uv = sm_p.tile([BL, H], F32, tag="uv", name="uv")
                        nc.vector.tensor_tensor(uv[:], c_prev[:], Fv[:], op=OP.mult)
                        nc.vector.tensor_tensor(c_new[:], c_new[:], uv[:], op=OP.add)
                    tanh_c = sm_p.tile([BL, H], F32, tag="tanhc", name="tanh_c")
                    nc.scalar.activation(tanh_c[:], c_new[:], AF.Tanh)

                    hout = state_p.tile([BL, H + 1], F32, tag="hout", name="hout")
                    og_view = gates[:].rearrange("p (l c) -> p l c", c=512)[:, :, 256:384]
                    nc.vector.tensor_tensor(
                        hout[:, 0:H].rearrange("p (l c) -> p l c", c=128),
                        og_view, tanh_c[:].rearrange("p (l c) -> p l c", c=128),
                        op=OP.mult)
                    # dist = 2/3 - (fm0+fm1)/3
                    nc.vector.tensor_tensor(sm[:, 27:28], sm[:, 12:13], sm[:, 13:14],
                                            op=OP.add)
                    nc.vector.tensor_scalar(
                        hout[:, H:H + 1], sm[:, 27:28],
                        scalar1=-1.0 / 3.0, scalar2=2.0 / 3.0, op0=OP.mult, op1=OP.add)

                    nc.sync.dma_start(d_hseq[(t + PAD) * BL:(t + PAD + 1) * BL, :],
                                      hout[:])
                    if debug_hseq:
                        nc.sync.dma_start(d_dbg_g[t * BL:(t + 1) * BL, :], gates[:])
                        nc.sync.dma_start(d_dbg_sm[t * BL:(t + 1) * BL, :], sm[:])

                    # ---- transpose h for next step ----
                    hT = [state_p.tile([128, BL], F32R, tag=f"hT{l}", name=f"hT{l}")
                          for l in range(3)]
                    for l in range(3):
                        base = (t % 2) * 192 + l * 64
                        dst = tr_ring[:, base:base + 64]
                        nc.tensor.transpose(dst, hout[:, l * 128:(l + 1) * 128], id64)
                        if l == 0:
                            nc.vector.tensor_copy(hT[l][:], dst)
                        else:
                            nc.scalar.copy(hT[l][:], dst)
                    hT_prev = hT
                    c_prev = c_new

            # ------------- phase 3: windowed head at t_b only -------------
            with (
                tc.tile_pool(name="postps", bufs=1, space="PSUM") as postps_p,
                tc.tile_pool(name="postps2", bufs=2, space="PSUM") as postps2_p,
            ):
                gath = [post_p.tile([128, H + 1], F32, tag=f"gath{j}", name=f"gath{j}")
                        for j in range(5)]
                for j in range(5):
                    nc.gpsimd.indirect_dma_start(
                        out=gath[j][:], out_offset=None, in_=d_hseq[:],
                        in_offset=bass.IndirectOffsetOnAxis(ap=gidx_sb[:, j:j + 1],
                                                            axis=0))

                # dist window [64, 10] -> cumsum -> softmax -> d_win
                ww = post_p.tile([BL, 16], F32, name="ww")
                cum = post_p.tile([BL, 64], F32, name="cum")
                for k in range(K):
                    j, par = k // 2, k % 2
                    nc.vector.tensor_copy(ww[:, k:k + 1],
                                          gath[j][par * 64:par * 64 + 64, H:H + 1])
                nc.vector.tensor_copy(cum[:, 0:1], ww[:, 0:1])
                nc.vector.tensor_tensor(cum[:, 1:10], ww[:, 1:10], ww[:, 0:9], op=OP.add)
                nc.vector.tensor_copy(cum[:, 16:18], cum[:, 0:2])
                nc.vector.tensor_tensor(cum[:, 18:26], cum[:, 2:10], cum[:, 0:8],
                                        op=OP.add)
                nc.vector.tensor_copy(cum[:, 32:36], cum[:, 16:20])
                nc.vector.tensor_tensor(cum[:, 36:42], cum[:, 20:26], cum[:, 16:22],
                                        op=OP.add)
                nc.vector.tensor_copy(cum[:, 48:56], cum[:, 32:40])
                nc.vector.tensor_tensor(cum[:, 56:58], cum[:, 40:42], cum[:, 32:34],
                                        op=OP.add)
                nmx = post_p.tile([BL, 4], F32, name="nmx")
                nc.vector.tensor_reduce(nmx[:, 0:1], cum[:, 48:58], axis=AX.X,
                                        op=OP.max, negate=True)
                dwin = post_p.tile([BL, 16], F32, name="dwin")
                nc.scalar.activation(dwin[:, 0:10], cum[:, 48:58], AF.Exp,
                                     bias=nmx[:, 0:1], scale=1.0)
                nc.vector.tensor_reduce(nmx[:, 1:2], dwin[:, 0:10], axis=AX.X,
                                        op=OP.add)
                nc.vector.reciprocal(nmx[:, 2:3], nmx[:, 1:2])
                nc.vector.tensor_scalar(dwin[:, 0:10], dwin[:, 0:10],
                                        scalar1=nmx[:, 2:3], scalar2=None, op0=OP.mult)

                # d_win -> [1, 640] k-major -> broadcast [128, 640]
                dT_ps = postps_p.tile([K, BL], F32, tag="dTps", name="dT_ps")
                nc.tensor.transpose(dT_ps[:], dwin[:, 0:K], id64)
                dT = post_p.tile([K, BL], F32, name="dT")
                nc.vector.tensor_copy(dT[:], dT_ps[:])
                nc.sync.dma_start(d_dscr[:], dT[:])
                dbc = post_p.tile([128, K * BL], F32, name="dbc")
                nc.gpsimd.dma_start(dbc[:], d_dscr[:].to_broadcast([128, K * BL]))

                # gathered h -> transposed chunks gathT[hc][h', k*64+b]
                gathT = [post_p.tile([128, K * BL], F32, tag=f"gathT{hc}",
                                     name=f"gathT{hc}") for hc in range(3)]
                for j in range(5):
                    for hc in range(3):
                        pt = postps2_p.tile([128, 128], F32, tag="postTp", name="pt2")
                        nc.tensor.transpose(pt[:], gath[j][:, hc * 128:(hc + 1) * 128],
                                            ident[:])
                        nc.scalar.copy(gathT[hc][:, j * 128:(j + 1) * 128], pt[:])

                # weighted taps (bf16) for conv + theme
                wg = [post_p.tile([128, K * BL], BF16, tag=f"wg{hc}", name=f"wg{hc}")
                      for hc in range(3)]
                for hc in range(3):
                    nc.vector.tensor_tensor(wg[hc][:], gathT[hc][:], dbc[:], op=OP.mult)

                # theme_in.T chunks = sum_k wg (scale_w already /10)
                thin = [post_p.tile([128, BL], F32, tag=f"thin{hc}", name=f"thin{hc}")
                        for hc in range(3)]
                for hc in range(3):
                    nc.vector.tensor_reduce(
                        thin[hc][:], wg[hc][:].rearrange("p (k b) -> p b k", b=BL),
                        axis=AX.X, op=OP.add)

                # MLP: u = thin @ scaleW + b -> relu -> v = ru @ rescaleW -> sigmoid
                u_ps = postps_p.tile([64, BL], F32, tag="ups", name="u_ps")
                for hc in range(3):
                    nc.tensor.matmul(u_ps[:], scaleW_sb[:, hc * 64:(hc + 1) * 64],
                                     thin[hc][:], start=(hc == 0), stop=(hc == 2))
                ru = post_p.tile([64, BL], F32, name="ru")
                nc.vector.tensor_scalar(ru[:], u_ps[:], scalar1=scaleb_sb[:, 0:1],
                                        scalar2=0.0, op0=OP.add, op1=OP.max)
                th = [post_p.tile([128, BL], F32, tag=f"th{oc}", name=f"th{oc}")
                      for oc in range(3)]
                for oc in range(3):
                    v_ps = postps_p.tile([128, BL], F32, tag="vps", name="v_ps")
                    nc.tensor.matmul(v_ps[:], rescaleW_sb[:, oc * 128:(oc + 1) * 128],
                                     ru[:], start=True, stop=True)
                    nc.scalar.activation(th[oc][:], v_ps[:], AF.Tanh,
                                         bias=rescb_sb[:, oc:oc + 1], scale=0.5)
                    nc.vector.tensor_scalar(th[oc][:], th[oc][:], scalar1=0.5,
                                            scalar2=0.5, op0=OP.mult, op1=OP.add)

                # conv.T[oc] = sum_{k,hc} convT_k_hc_oc.T @ wg[hc][:, k-slice]
                rnnT = [post_p.tile([128, BL], F32, tag=f"rnnT{oc}", name=f"rnnT{oc}")
                        for oc in range(3)]
                for oc in range(3):
                    cv_ps = postps2_p.tile([128, BL], F32, tag="cvps", name="cv_ps")
                    n = 0
                    for k in range(K):
                        for hc in range(3):
                            off = ((k * 3 + hc) * 3 + oc) * CH
                            nc.tensor.matmul(
                                cv_ps[:], convT_sb[:, off:off + CH],
                                wg[hc][:, k * BL:(k + 1) * BL],
                                start=(n == 0), stop=(n == 3 * K - 1))
                            n += 1
                    # (conv + conv_b) * theme + h_sel
                    nc.vector.tensor_scalar(rnnT[oc][:], cv_ps[:],
                                            scalar1=convb_sb[:, oc:oc + 1],
                                            scalar2=None, op0=OP.add)
                    nc.vector.tensor_tensor(rnnT[oc][:], rnnT[oc][:], th[oc][:],
                                            op=OP.mult)
                    nc.vector.tensor_tensor(rnnT[oc][:], rnnT[oc][:],
                                            gathT[oc][:, (K - 1) * BL:K * BL],
                                            op=OP.add)

                # output projection + bias
                o_ps = postps_p.tile([BL, LAB], F32, tag="ops", name="o_ps")
                for hc in range(3):
                    nc.tensor.matmul(o_ps[:], rnnT[hc][:],
                                     outW_sb[:, hc * LAB:(hc + 1) * LAB],
                                     start=(hc == 0), stop=(hc == 2))
                ofin = post_p.tile([BL, LAB], F32, name="ofin")
                nc.vector.tensor_tensor(ofin[:], o_ps[:], outb_sb[:], op=OP.add)
                nc.sync.dma_start(d_out[:], ofin[:])

    _split_drain_waits(nc)
    return nc


def _split_drain_waits(nc, limit=1):
    """Workaround: this walrus rejects instructions carrying more than
    `limit` semaphore waits; hoist excess waits onto NoOps just before."""
    n = 0
    for fn in nc.m.functions:
        for bb in fn.blocks:
            new_insts = []
            for inst in bb.instructions:
                si = inst.sync_info
                if si and si.on_wait and len(si.on_wait) > limit:
                    waits = list(si.on_wait)
                    for w in waits[limit:]:
                        n += 1
                        nop = mybir.InstNoOp(name=f"I-dsplit-{n}", ins=[], outs=[])
                        nop.engine = inst.engine
                        nop.sync_info = mybir.SyncInfo(on_wait=[w], on_update=[])
                        new_insts.append(nop)
                    inst.sync_info = mybir.SyncInfo(on_wait=waits[:limit],
                                                    on_update=list(si.on_update))
                new_insts.append(inst)
            bb.instructions = new_insts
    return n


def _make_inmaps(inputs, t_steps=T, ncores=NCORES):
    X = np.asarray(inputs["X"], np.float32)
    v_lengths = np.asarray(inputs["v_lengths"]).astype(np.int64)
    shared = _prep_shared(
        np.asarray(inputs["kernel_w"], np.float32), np.asarray(inputs["kernel_b"], np.float32),
        np.asarray(inputs["rec_w"], np.float32), np.asarray(inputs["rec_b"], np.float32),
        np.asarray(inputs["scale_w"], np.float32), np.asarray(inputs["scale_b"], np.float32),
        np.asarray(inputs["rescale_w"], np.float32), np.asarray(inputs["rescale_b"], np.float32),
        np.asarray(inputs["conv_w"], np.float32), np.asarray(inputs["conv_b"], np.float32),
        np.asarray(inputs["out_w"], np.float32), np.asarray(inputs["out_b"], np.float32))
    in_maps = []
    for c in range(ncores):
        bs = slice(c * BL, (c + 1) * BL)
        vl = v_lengths[bs]
        gidx = np.zeros((128, 5), np.int32)
        for p in range(128):
            for j in range(5):
                b = p % 64
                k = 2 * j + p // 64
                tb = int(vl[b]) - 1
                gidx[p, j] = (tb + k) * BL + b
        m = dict(shared)
        m["x"] = np.ascontiguousarray(X[bs, :t_steps, :])
        m["gidx"] = gidx
        in_maps.append(m)
    return in_maps


_NC_CACHE = {}


def kernel(**inputs) -> np.ndarray:
    t_steps = T
    if t_steps not in _NC_CACHE:
        _NC_CACHE[t_steps] = build_nc(t_steps)
    nc = _NC_CACHE[t_steps]
    in_maps = _make_inmaps(inputs, t_steps)
    res = run_bass_kernel_spmd(nc, in_maps, list(range(NCORES)))
    out = np.concatenate([res.results[c]["cur_out"] for c in range(NCORES)], axis=0)
    return out.astype(np.float32)



# revision 5
# speedup vs baseline: 1.1954x; 1.1954x over previous
"""Trainium2 Bass kernel for nn_BuildModel_3796751089795 (ON-LSTM-style RNN).

Model per reference:
  - sequential ON-LSTM cell over T=128 steps (cumax master gates L=3, CH=128)
  - per-step sliding-window (K=10) "theme"/"conv" head
  - output projection + per-batch-row selection at t = v_lengths[b]-1

Key algorithmic facts exploited:
  * only cur_output[b] = rnn[t_b, b] @ out_w + out_b is needed (t_b =
    v_lengths[b]-1), so the windowed conv/theme head is computed ONLY at t_b
    per batch row (gathered from the stored h/dist sequence), not at all T.
  * the x_t @ kernel_w part of the gate preactivation is computed inside the
    step loop by accumulating into the same PSUM tile as the h @ rec_w part
    (2 F-chunk matmuls against an on-chip transposed copy of X).
  * sigmoid(x) = 0.5*tanh(0.5x)+0.5 so every gate nonlinearity uses the one
    ACT table set that also contains exp (no per-step table switches).
  * local_dis softmax is computed on the gathered 10-tap dist window only.

Sharding: data-parallel over batch, B=512 -> 8 cores x 64 rows.

Self-contained: hardcodes all shapes; no file reads.
"""

import numpy as np
import ml_dtypes

import concourse.bass as bass
import concourse.tile as tile
from concourse import mybir
from concourse.bass_utils import run_bass_kernel_spmd
from concourse.masks import make_identity

F32 = mybir.dt.float32
F32R = mybir.dt.float32r
BF16 = mybir.dt.bfloat16
I32 = mybir.dt.int32
AF = mybir.ActivationFunctionType
OP = mybir.AluOpType
AX = mybir.AxisListType

B, T, F, H, L, K, LAB = 512, 128, 256, 384, 3, 10, 25
CH = H // L            # 128
GATES = 4 * H + 2 * L  # 1542
NCORES = 8
BL = B // NCORES       # 64 batch rows per core
PAD = K - 1            # 9 zero timesteps of h-prefix for window taps t<0

# reordered gate-column layout (see _prep_shared):
#   cols [0:1536): 3 blocks of 512 = [f_l(128) i_l(128) o_l(128) ci_l(128)]
#   cols [1536:1542): fm(3) im(3) preactivations
NG = 1536


def _gate_perm_scale():
    """Return (perm, scale): new_W[:, j] = old_W[:, perm[j]] * scale[j]."""
    perm = np.zeros(GATES, np.int64)
    scale = np.ones(GATES, np.float32)
    for l in range(L):
        base = l * 4 * CH
        for gi in range(4):   # f, i, o, ci
            perm[base + gi * CH: base + (gi + 1) * CH] = np.arange(
                2 * L + gi * H + l * CH, 2 * L + gi * H + (l + 1) * CH)
        # f/i/o go through sigmoid-via-tanh: pre-scale by 0.5; ci stays
        scale[base: base + 3 * CH] = 0.5
    perm[NG:] = np.arange(2 * L)
    return perm, scale


def _prep_shared(kernel_w, kernel_b, rec_w, rec_b, scale_w, scale_b,
                 rescale_w, rescale_b, conv_w, conv_b, out_w, out_b):
    """Host-side weight preprocessing (replicated across cores).

    hseq stores hs = 2*h (h computed as (tanh(o/2)+1)*tanh(c) in one fused
    op); compensated by wrec *= 0.5 (recurrence), scale_w *= 0.5 (theme MLP
    input), conv_b *= 2 and out_w *= 0.5 (rnn' = 2*rnn)."""
    scale_w = scale_w * 0.5
    conv_b = conv_b * 2.0
    out_w = out_w * 0.5
    perm, colscale = _gate_perm_scale()

    def reorder(v):  # v [..., GATES]
        return (v[..., perm] * colscale).astype(np.float32)

    wpre = reorder(kernel_w[:F])                     # [256, 1542]
    wrec = reorder(rec_w[:H] * 0.5)                  # [384, 1542]; hs=2h comp.
    cb = reorder(kernel_b + rec_b + kernel_w[F] + rec_w[H])[None, :]   # [1,1542]
    trow = reorder(kernel_w[F] + rec_w[H])           # [1542]
    trow_rep = np.tile(trow[None, :], (BL, 1)).astype(np.float32)      # [64,1542]

    # conv_w [O=384, Hin=384, K=10] -> convT [128(h'), (k,hc,oc,o)] bf16
    convT = np.zeros((CH, K * 3 * 3 * CH), np.float32)
    for k in range(K):
        for hc in range(3):
            for oc in range(3):
                blk = conv_w[oc * CH:(oc + 1) * CH, hc * CH:(hc + 1) * CH, k].T
                off = ((k * 3 + hc) * 3 + oc) * CH
                convT[:, off:off + CH] = blk
    convT = convT.astype(ml_dtypes.bfloat16)

    scaleW = np.zeros((CH, 3 * 64), np.float32)      # lhsT chunks [h',feat]
    for hc in range(3):
        scaleW[:, hc * 64:(hc + 1) * 64] = scale_w[hc * CH:(hc + 1) * CH, :] / 10.0
    scalebT = scale_b.reshape(64, 1).astype(np.float32)
    rescaleW = rescale_w.astype(np.float32)          # [64, 384] = [K, (oc,o)]
    rescbT = np.zeros((CH, 3), np.float32)
    convbT = np.zeros((CH, 3), np.float32)
    outW = np.zeros((CH, 3 * LAB), np.float32)
    for oc in range(3):
        rescbT[:, oc] = 0.5 * rescale_b[oc * CH:(oc + 1) * CH]
        convbT[:, oc] = conv_b[oc * CH:(oc + 1) * CH]
        outW[:, oc * LAB:(oc + 1) * LAB] = out_w[oc * CH:(oc + 1) * CH, :]
    outb_rep = np.tile(out_b[None, :], (BL, 1)).astype(np.float32)

    return dict(
        wpre0=np.ascontiguousarray(wpre[:128]), wpre1=np.ascontiguousarray(wpre[128:]),
        wrec0=np.ascontiguousarray(wrec[:128]), wrec1=np.ascontiguousarray(wrec[128:256]),
        wrec2=np.ascontiguousarray(wrec[256:]),
        cbw=cb.astype(np.float32), trow_rep=trow_rep,
        convT=convT, scaleW=scaleW, scalebT=scalebT,
        rescaleW=rescaleW, rescbT=rescbT, convbT=convbT,
        outW=outW, outb_rep=outb_rep,
        ones1=np.ones((1, BL), np.float32),
    )


def build_nc(t_steps=T, debug_hseq=False):
    """Build the Bass module for one core (B-shard of 64 rows)."""
    nc = bass.Bass()
    ROWS = BL * t_steps
    HS_ROWS = (t_steps + PAD) * BL

    # ---------------- I/O ----------------
    d_x = nc.dram_tensor("x", [BL, t_steps, F], F32, kind="ExternalInput")
    d_wpre = [nc.dram_tensor(f"wpre{i}", [128, GATES], F32R, kind="ExternalInput")
              for i in range(2)]
    d_wrec = [nc.dram_tensor(f"wrec{i}", [128, GATES], F32R, kind="ExternalInput")
              for i in range(3)]
    d_cbw = nc.dram_tensor("cbw", [1, GATES], F32R, kind="ExternalInput")
    d_trow = nc.dram_tensor("trow_rep", [BL, GATES], F32, kind="ExternalInput")
    d_convT = nc.dram_tensor("convT", [CH, K * 9 * CH], BF16, kind="ExternalInput")
    d_scaleW = nc.dram_tensor("scaleW", [CH, 192], F32, kind="ExternalInput")
    d_scaleb = nc.dram_tensor("scalebT", [64, 1], F32, kind="ExternalInput")
    d_rescaleW = nc.dram_tensor("rescaleW", [64, H], F32, kind="ExternalInput")
    d_rescb = nc.dram_tensor("rescbT", [CH, 3], F32, kind="ExternalInput")
    d_convb = nc.dram_tensor("convbT", [CH, 3], F32, kind="ExternalInput")
    d_outW = nc.dram_tensor("outW", [CH, 3 * LAB], F32, kind="ExternalInput")
    d_outb = nc.dram_tensor("outb_rep", [BL, LAB], F32, kind="ExternalInput")
    d_ones1 = nc.dram_tensor("ones1", [1, BL], F32R, kind="ExternalInput")
    d_gidx = nc.dram_tensor("gidx", [128, 5], I32, kind="ExternalInput")

    hseq_kind = "ExternalOutput" if debug_hseq else "Internal"
    d_hseq = nc.dram_tensor("hseq", [HS_ROWS, H + 1], F32, kind=hseq_kind)
    d_out = nc.dram_tensor("cur_out", [BL, LAB], F32, kind="ExternalOutput")
    d_dscr = nc.dram_tensor("dscr", [1, K * BL], F32)
    d_dbg_g = d_dbg_sm = None
    if debug_hseq:
        d_dbg_g = nc.dram_tensor("dbg_g", [t_steps * BL, NG], F32, kind="ExternalOutput")
        d_dbg_sm = nc.dram_tensor("dbg_sm", [t_steps * BL, 32], F32, kind="ExternalOutput")

    with tile.TileContext(nc) as tc:
        with (
            tc.tile_pool(name="singles", bufs=1) as singles,
            tc.tile_pool(name="post", bufs=1) as post_p,
        ):
            # ------------- load constants -------------
            ident = singles.tile([128, 128], F32)
            make_identity(nc, ident[:])
            id64 = ident[0:64, 0:64]
            wpre_sb = [singles.tile([128, GATES], F32R, tag=f"wpre{i}", name=f"wpre{i}")
                       for i in range(2)]
            wrec_sb = [singles.tile([128, GATES], F32R, tag=f"wrec{i}", name=f"wrec{i}")
                       for i in range(3)]
            for i in range(2):
                nc.sync.dma_start(wpre_sb[i][:], d_wpre[i][:])
            for i in range(3):
                nc.sync.dma_start(wrec_sb[i][:], d_wrec[i][:])
            cbw_sb = singles.tile([1, GATES], F32R)
            nc.sync.dma_start(cbw_sb[:], d_cbw[:])
            trow_sb = singles.tile([BL, GATES], F32)
            nc.sync.dma_start(trow_sb[:], d_trow[:])
            convT_sb = singles.tile([CH, K * 9 * CH], BF16)
            nc.sync.dma_start(convT_sb[:], d_convT[:])
            scaleW_sb = singles.tile([CH, 192], F32)
            nc.sync.dma_start(scaleW_sb[:], d_scaleW[:])
            scaleb_sb = singles.tile([64, 1], F32)
            nc.sync.dma_start(scaleb_sb[:], d_scaleb[:])
            rescaleW_sb = singles.tile([64, H], F32)
            nc.sync.dma_start(rescaleW_sb[:], d_rescaleW[:])
            rescb_sb = singles.tile([CH, 3], F32)
            nc.sync.dma_start(rescb_sb[:], d_rescb[:])
            convb_sb = singles.tile([CH, 3], F32)
            nc.sync.dma_start(convb_sb[:], d_convb[:])
            outW_sb = singles.tile([CH, 3 * LAB], F32)
            nc.sync.dma_start(outW_sb[:], d_outW[:])
            outb_sb = singles.tile([BL, LAB], F32)
            nc.sync.dma_start(outb_sb[:], d_outb[:])
            ones1_sb = singles.tile([1, BL], F32R)
            nc.sync.dma_start(ones1_sb[:], d_ones1[:])
            gidx_sb = singles.tile([128, 5], I32)
            nc.sync.dma_start(gidx_sb[:], d_gidx[:])

            # zero the h-prefix rows of hseq
            zrow = singles.tile([128, H + 1], F32)
            nc.vector.memset(zrow[:], 0.0)
            zr = PAD * BL  # 576
            for r0 in range(0, zr, 128):
                n = min(128, zr - r0)
                nc.sync.dma_start(d_hseq[r0:r0 + n, :], zrow[:n, :])

            # ------------- phase 1: X -> XT (F-major) -------------
            # XT[fc][f, t*64+b] = X[b, t, f]
            xt_sb = [singles.tile([128, ROWS], F32R, tag=f"xt{i}", name=f"xt{i}")
                     for i in range(2)]
            x_tmaj = d_x[:].rearrange("b t f -> t b f")
            with (
                tc.tile_pool(name="xrow", bufs=4) as xrow_p,
                tc.tile_pool(name="trps", bufs=4, space="PSUM") as trps_p,
            ):
                for rt in range(ROWS // 128):
                    xr = xrow_p.tile([128, F], F32, tag="xrow", name="xr")
                    t0 = rt * 2
                    nc.sync.dma_start(xr[0:64, :], x_tmaj[t0, :, :])
                    nc.sync.dma_start(xr[64:128, :], x_tmaj[t0 + 1, :, :])
                    for fc in range(2):
                        pt = trps_p.tile([128, 128], F32, tag="xtp", name="pt")
                        nc.tensor.transpose(pt[:], xr[:, fc * 128:(fc + 1) * 128],
                                            ident[:])
                        nc.scalar.copy(xt_sb[fc][:, rt * 128:(rt + 1) * 128], pt[:])

            # ------------- phase 2: recurrence -------------
            # Per step (steady state), issue order chosen so each engine's
            # in-order queue never blocks the critical chain:
            #   PE:   [slot h-mms][tile h-mms 0,1,2][x-mms for t+1][T0 T1 T2]
            #   Act:  [exp][tanh0][tanh1][tanh2][tc0][tc1][tc2]
            #   DVE:  [sm chain][cell1][cell2][h0 h1 h2][dist]
            #   Pool: [cell0][hT copy]
            # Sigmoids are folded into per-l scalars: F = a*tf+b, I = a*ti+d
            # with a = s1/2, b = fm-a, d = im-a (tf/ti = tanh(pre/2)).
            # h is stored as hs = 2h = (to+1)*tanh(c)  (one fused stt op);
            # weight compensation happens host-side in _prep_shared.
            with (
                tc.tile_pool(name="xo", bufs=2, space="PSUM") as xo_p,
                tc.tile_pool(name="smallps", bufs=1, space="PSUM") as smallps_p,
                tc.tile_pool(name="gates", bufs=2) as gates_p,
                tc.tile_pool(name="state", bufs=2) as state_p,
                tc.tile_pool(name="wk", bufs=2) as wk_p,
            ):
                fmim_ring = smallps_p.tile([BL, 512], F32, tag="fmim", name="fmim")
                tr_ring = smallps_p.tile([128, 384], F32, tag="trring", name="trring")
                sm_ring = singles.tile([BL, 128], F32)
                nc.vector.memset(sm_ring[:], 1.0)   # cols 12,13 stay 1 forever

                def slot_ap(t):
                    o = (t % 64) * 8
                    return fmim_ring[:, o:o + 6]

                def issue_x(t, xo_tiles, stop):
                    sl = slot_ap(t)
                    nc.tensor.matmul(sl, ones1_sb[:], cbw_sb[:, NG:],
                                     start=True, stop=False)
                    for fc in range(2):
                        nc.tensor.matmul(
                            sl, xt_sb[fc][:, t * BL:(t + 1) * BL],
                            wpre_sb[fc][:, NG:], start=False,
                            stop=(stop and fc == 1))
                    for nt in range(3):
                        cs = slice(nt * 512, (nt + 1) * 512)
                        nc.tensor.matmul(xo_tiles[nt][:], ones1_sb[:],
                                         cbw_sb[:, cs], start=True, stop=False)
                        for fc in range(2):
                            nc.tensor.matmul(
                                xo_tiles[nt][:],
                                xt_sb[fc][:, t * BL:(t + 1) * BL],
                                wpre_sb[fc][:, cs], start=False,
                                stop=(stop and fc == 1))

                def new_xo(t):
                    return [xo_p.tile([BL, 512], F32, tag=f"xo{nt}",
                                      name=f"xo{nt}_{t}") for nt in range(3)]

                xo_cur = new_xo(0)
                issue_x(0, xo_cur, stop=True)

                hT_prev = None   # [128, 192] f32r, chunk hc at cols hc*64
                c_prev = None    # [64, 384]

                for t in range(t_steps):
                    slot = slot_ap(t)
                    # ---- h matmuls (close the accumulation groups) ----
                    if t > 0:
                        for hc in range(3):
                            nc.tensor.matmul(
                                slot, hT_prev[:, hc * 64:(hc + 1) * 64],
                                wrec_sb[hc][:, NG:], start=False, stop=(hc == 2))
                        for nt in range(3):
                            cs = slice(nt * 512, (nt + 1) * 512)
                            for hc in range(3):
                                nc.tensor.matmul(
                                    xo_cur[nt][:],
                                    hT_prev[:, hc * 64:(hc + 1) * 64],
                                    wrec_sb[hc][:, cs], start=False,
                                    stop=(hc == 2))
                    else:
                        # t=0: Tint=0 -> remove trow part of the bias row
                        for nt in range(3):
                            cs = slice(nt * 512, (nt + 1) * 512)
                            nc.vector.tensor_tensor(
                                xo_cur[nt][:], xo_cur[nt][:], trow_sb[:, cs],
                                op=OP.subtract)
                        nc.vector.tensor_tensor(slot, slot, trow_sb[:, NG:],
                                                op=OP.subtract)

                    # ---- sm chain: exp (Act) + DVE small ops ----
                    # slice cols: 0:6 e (1->u01, 4->u45), 6:8 sums, 8:10 r,
                    # 10:13 fm trio, 13:16 im trio (cols 12,13 preset to 1),
                    # 16:19 s1, 19:22 a, 22:25 b, 25:28 d, 28 fm0+fm1
                    so = (t % 4) * 32
                    sm = sm_ring[:, so:so + 32]
                    nc.scalar.activation(sm[:, 0:6], slot, AF.Exp)
                    nc.vector.tensor_reduce(
                        sm[:, 6:8], sm[:, 0:6].rearrange("p (a b) -> p a b", b=3),
                        axis=AX.X, op=OP.add)
                    nc.vector.tensor_tensor(sm[:, 1:2], sm[:, 1:2], sm[:, 0:1],
                                            op=OP.add)   # u01
                    nc.vector.tensor_tensor(sm[:, 4:5], sm[:, 4:5], sm[:, 5:6],
                                            op=OP.add)   # u45
                    nc.vector.reciprocal(sm[:, 8:10], sm[:, 6:8])
                    nc.vector.tensor_scalar(sm[:, 10:12], sm[:, 0:2],
                                            scalar1=sm[:, 8:9], scalar2=None,
                                            op0=OP.mult)   # fm0, fm1
                    nc.vector.tensor_scalar(sm[:, 14:16], sm[:, 4:6],
                                            scalar1=sm[:, 9:10], scalar2=None,
                                            op0=OP.mult)   # im1, im2
                    nc.vector.tensor_tensor(sm[:, 16:19], sm[:, 10:13],
                                            sm[:, 13:16], op=OP.mult)   # s1
                    nc.vector.tensor_scalar(sm[:, 19:22], sm[:, 16:19],
                                            scalar1=0.5, scalar2=None,
                                            op0=OP.mult)   # a
                    nc.vector.scalar_tensor_tensor(
                        sm[:, 22:25], sm[:, 16:19], -0.5, sm[:, 10:13],
                        op0=OP.mult, op1=OP.add)            # b = fm - a
                    nc.vector.scalar_tensor_tensor(
                        sm[:, 25:28], sm[:, 16:19], -0.5, sm[:, 13:16],
                        op0=OP.mult, op1=OP.add)            # d = im - a

                    # ---- x-part matmuls for t+1 (keeps PE fed) ----
                    if t + 1 < t_steps:
                        xo_next = new_xo(t + 1)
                        issue_x(t + 1, xo_next, stop=False)
                    else:
                        xo_next = None

                    # ---- per-tile: tanh, then cell update ----
                    gates = [gates_p.tile([BL, 512], F32, tag=f"g{nt}",
                                          name=f"g{nt}_{t}") for nt in range(3)]
                    c_new = state_p.tile([BL, H], F32, tag="c", name=f"c_{t}")
                    tcs = wk_p.tile([BL, H], F32, tag="tcs", name=f"tcs_{t}")
                    for nt in range(3):
                        nc.scalar.activation(gates[nt][:], xo_cur[nt][:], AF.Tanh)
                    for nt in range(3):
                        g = gates[nt]
                        blk = slice(nt * 128, (nt + 1) * 128)
                        a_l = sm[:, 19 + nt:20 + nt]
                        b_l = sm[:, 22 + nt:23 + nt]
                        d_l = sm[:, 25 + nt:26 + nt]
                        eng = nc.gpsimd if nt == 0 else nc.vector
                        Fv = wk_p.tile([BL, 128], F32, tag=f"Fv{nt}",
                                       name=f"Fv{nt}_{t}")
                        Iv = wk_p.tile([BL, 128], F32, tag=f"Iv{nt}",
                                       name=f"Iv{nt}_{t}")
                        uv = wk_p.tile([BL, 128], F32, tag=f"uv{nt}",
                                       name=f"uv{nt}_{t}")
                        eng.tensor_scalar(Iv[:], g[:, 128:256], scalar1=a_l,
                                          scalar2=d_l, op0=OP.mult, op1=OP.add)
                        if t > 0:
                            eng.tensor_scalar(Fv[:], g[:, 0:128], scalar1=a_l,
                                              scalar2=b_l, op0=OP.mult, op1=OP.add)
                            eng.tensor_tensor(uv[:], Fv[:], c_prev[:, blk],
                                              op=OP.mult)
                            eng.tensor_tensor(c_new[:, blk], Iv[:], g[:, 384:512],
                                              op=OP.mult)
                            eng.tensor_tensor(c_new[:, blk], c_new[:, blk], uv[:],
                                              op=OP.add)
                        else:
                            eng.tensor_tensor(c_new[:, blk], Iv[:], g[:, 384:512],
                                              op=OP.mult)
                    # tanh(c) per tile on Act (queue order: after the 3 tanhs)
                    for nt in range(3):
                        blk = slice(nt * 128, (nt + 1) * 128)
                        nc.scalar.activation(tcs[:, blk], c_new[:, blk], AF.Tanh)

                    # ---- hs = (to+1)*tanh(c) = 2h ; dist ----
                    hout = state_p.tile([BL, H + 1], F32, tag="hout",
                                        name=f"hout_{t}")
                    for nt in range(3):
                        blk = slice(nt * 128, (nt + 1) * 128)
                        nc.vector.scalar_tensor_tensor(
                            hout[:, blk], gates[nt][:, 256:384], 1.0, tcs[:, blk],
                            op0=OP.add, op1=OP.mult)
                    nc.vector.tensor_tensor(sm[:, 28:29], sm[:, 10:11],
                                            sm[:, 11:12], op=OP.add)
                    nc.vector.tensor_scalar(
                        hout[:, H:H + 1], sm[:, 28:29],
                        scalar1=-1.0 / 3.0, scalar2=2.0 / 3.0,
                        op0=OP.mult, op1=OP.add)

                    nc.sync.dma_start(d_hseq[(t + PAD) * BL:(t + PAD + 1) * BL, :],
                                      hout[:])
                    if debug_hseq:
                        nc.sync.dma_start(d_dbg_sm[t * BL:(t + 1) * BL, :],
                                          sm_ring[:, 0:32])

                    # ---- transpose h for next step; one PSUM->SBUF copy ----
                    if t + 1 < t_steps:
                        base = (t % 2) * 192
                        for nt in range(3):
                            nc.tensor.transpose(
                                tr_ring[:, base + nt * 64:base + nt * 64 + 64],
                                hout[:, nt * 128:(nt + 1) * 128], id64)
                        hT_new = state_p.tile([128, 192], F32R, tag="hT",
                                              name=f"hT_{t}")
                        nc.vector.tensor_copy(hT_new[:],
                                              tr_ring[:, base:base + 192])
                        hT_prev = hT_new
                    xo_cur = xo_next
                    c_prev = c_new

            # ------------- phase 3: windowed head at t_b only -------------
            with (
                tc.tile_pool(name="postps", bufs=1, space="PSUM") as postps_p,
                tc.tile_pool(name="postps2", bufs=2, space="PSUM") as postps2_p,
            ):
                gath = [post_p.tile([128, H + 1], F32, tag=f"gath{j}", name=f"gath{j}")
                        for j in range(5)]
                for j in range(5):
                    nc.gpsimd.indirect_dma_start(
                        out=gath[j][:], out_offset=None, in_=d_hseq[:],
                        in_offset=bass.IndirectOffsetOnAxis(ap=gidx_sb[:, j:j + 1],
                                                            axis=0))

                # dist window [64, 10] -> cumsum -> softmax -> d_win
                ww = post_p.tile([BL, 16], F32, name="ww")
                cum = post_p.tile([BL, 64], F32, name="cum")
                for k in range(K):
                    j, par = k // 2, k % 2
                    nc.vector.tensor_copy(ww[:, k:k + 1],
                                          gath[j][par * 64:par * 64 + 64, H:H + 1])
                nc.vector.tensor_copy(cum[:, 0:1], ww[:, 0:1])
                nc.vector.tensor_tensor(cum[:, 1:10], ww[:, 1:10], ww[:, 0:9], op=OP.add)
                nc.vector.tensor_copy(cum[:, 16:18], cum[:, 0:2])
                nc.vector.tensor_tensor(cum[:, 18:26], cum[:, 2:10], cum[:, 0:8],
                                        op=OP.add)
                nc.vector.tensor_copy(cum[:, 32:36], cum[:, 16:20])
                nc.vector.tensor_tensor(cum[:, 36:42], cum[:, 20:26], cum[:, 16:22],
                                        op=OP.add)
                nc.vector.tensor_copy(cum[:, 48:56], cum[:, 32:40])
                nc.vector.tensor_tensor(cum[:, 56:58], cum[:, 40:42], cum[:, 32:34],
                                        op=OP.add)
                nmx = post_p.tile([BL, 4], F32, name="nmx")
                nc.vector.tensor_reduce(nmx[:, 0:1], cum[:, 48:58], axis=AX.X,
                                        op=OP.max, negate=True)
                dwin = post_p.tile([BL, 16], F32, name="dwin")
                nc.scalar.activation(dwin[:, 0:10], cum[:, 48:58], AF.Exp,
                                     bias=nmx[:, 0:1], scale=1.0)
                nc.vector.tensor_reduce(nmx[:, 1:2], dwin[:, 0:10], axis=AX.X,
                                        op=OP.add)
                nc.vector.reciprocal(nmx[:, 2:3], nmx[:, 1:2])
                nc.vector.tensor_scalar(dwin[:, 0:10], dwin[:, 0:10],
                                        scalar1=nmx[:, 2:3], scalar2=None, op0=OP.mult)

                # d_win -> [1, 640] k-major -> broadcast [128, 640]
                dT_ps = postps_p.tile([K, BL], F32, tag="dTps", name="dT_ps")
                nc.tensor.transpose(dT_ps[:], dwin[:, 0:K], id64)
                dT = post_p.tile([K, BL], F32, name="dT")
                nc.vector.tensor_copy(dT[:], dT_ps[:])
                nc.sync.dma_start(d_dscr[:], dT[:])
                dbc = post_p.tile([128, K * BL], F32, name="dbc")
                nc.gpsimd.dma_start(dbc[:], d_dscr[:].to_broadcast([128, K * BL]))

                # gathered h -> transposed chunks gathT[hc][h', k*64+b]
                gathT = [post_p.tile([128, K * BL], F32, tag=f"gathT{hc}",
                                     name=f"gathT{hc}") for hc in range(3)]
                for j in range(5):
                    for hc in range(3):
                        pt = postps2_p.tile([128, 128], F32, tag="postTp", name="pt2")
                        nc.tensor.transpose(pt[:], gath[j][:, hc * 128:(hc + 1) * 128],
                                            ident[:])
                        nc.scalar.copy(gathT[hc][:, j * 128:(j + 1) * 128], pt[:])

                # weighted taps (bf16) for conv + theme
                wg = [post_p.tile([128, K * BL], BF16, tag=f"wg{hc}", name=f"wg{hc}")
                      for hc in range(3)]
                for hc in range(3):
                    nc.vector.tensor_tensor(wg[hc][:], gathT[hc][:], dbc[:], op=OP.mult)

                # theme_in.T chunks = sum_k wg (scale_w already /10)
                thin = [post_p.tile([128, BL], F32, tag=f"thin{hc}", name=f"thin{hc}")
                        for hc in range(3)]
                for hc in range(3):
                    nc.vector.tensor_reduce(
                        thin[hc][:], wg[hc][:].rearrange("p (k b) -> p b k", b=BL),
                        axis=AX.X, op=OP.add)

                # MLP: u = thin @ scaleW + b -> relu -> v = ru @ rescaleW -> sigmoid
                u_ps = postps_p.tile([64, BL], F32, tag="ups", name="u_ps")
                for hc in range(3):
                    nc.tensor.matmul(u_ps[:], scaleW_sb[:, hc * 64:(hc + 1) * 64],
                                     thin[hc][:], start=(hc == 0), stop=(hc == 2))
                ru = post_p.tile([64, BL], F32, name="ru")
                nc.vector.tensor_scalar(ru[:], u_ps[:], scalar1=scaleb_sb[:, 0:1],
                                        scalar2=0.0, op0=OP.add, op1=OP.max)
                th = [post_p.tile([128, BL], F32, tag=f"th{oc}", name=f"th{oc}")
                      for oc in range(3)]
                for oc in range(3):
                    v_ps = postps_p.tile([128, BL], F32, tag="vps", name="v_ps")
                    nc.tensor.matmul(v_ps[:], rescaleW_sb[:, oc * 128:(oc + 1) * 128],
                                     ru[:], start=True, stop=True)
                    nc.scalar.activation(th[oc][:], v_ps[:], AF.Tanh,
                                         bias=rescb_sb[:, oc:oc + 1], scale=0.5)
                    nc.vector.tensor_scalar(th[oc][:], th[oc][:], scalar1=0.5,
                                            scalar2=0.5, op0=OP.mult, op1=OP.add)

                # conv.T[oc] = sum_{k,hc} convT_k_hc_oc.T @ wg[hc][:, k-slice]
                rnnT = [post_p.tile([128, BL], F32, tag=f"rnnT{oc}", name=f"rnnT{oc}")
                        for oc in range(3)]
                for oc in range(3):
                    cv_ps = postps2_p.tile([128, BL], F32, tag="cvps", name="cv_ps")
                    n = 0
                    for k in range(K):
                        for hc in range(3):
                            off = ((k * 3 + hc) * 3 + oc) * CH
                            nc.tensor.matmul(
                                cv_ps[:], convT_sb[:, off:off + CH],
                                wg[hc][:, k * BL:(k + 1) * BL],
                                start=(n == 0), stop=(n == 3 * K - 1))
                            n += 1
                    # (conv + conv_b) * theme + h_sel
                    nc.vector.tensor_scalar(rnnT[oc][:], cv_ps[:],
                                            scalar1=convb_sb[:, oc:oc + 1],
                                            scalar2=None, op0=OP.add)
                    nc.vector.tensor_tensor(rnnT[oc][:], rnnT[oc][:], th[oc][:],
                                            op=OP.mult)
                    nc.vector.tensor_tensor(rnnT[oc][:], rnnT[oc][:],
                                            gathT[oc][:, (K - 1) * BL:K * BL],
                                            op=OP.add)

                # output projection + bias
                o_ps = postps_p.tile([BL, LAB], F32, tag="ops", name="o_ps")
                for hc in range(3):
                    nc.tensor.matmul(o_ps[:], rnnT[hc][:],
                                     outW_sb[:, hc * LAB:(hc + 1) * LAB],
                                     start=(hc == 0), stop=(hc == 2))
                ofin = post_p.tile([BL, LAB], F32, name="ofin")
                nc.vector.tensor_tensor(ofin[:], o_ps[:], outb_sb[:], op=OP.add)
                nc.sync.dma_start(d_out[:], ofin[:])

    _split_drain_waits(nc)
    return nc


def _split_drain_waits(nc, limit=1):
    """Workaround: this walrus rejects instructions carrying more than
    `limit` semaphore waits; hoist excess waits onto NoOps just before."""
    n = 0
    for fn in nc.m.functions:
        for bb in fn.blocks:
            new_insts = []
            for inst in bb.instructions:
                si = inst.sync_info
                if si and si.on_wait and len(si.on_wait) > limit:
                    waits = list(si.on_wait)
                    for w in waits[limit:]:
                        n += 1
                        nop = mybir.InstNoOp(name=f"I-dsplit-{n}", ins=[], outs=[])
                        nop.engine = inst.engine
                        nop.sync_info = mybir.SyncInfo(on_wait=[w], on_update=[])
                        new_insts.append(nop)
                    inst.sync_info = mybir.SyncInfo(on_wait=waits[:limit],
                                                    on_update=list(si.on_update))
                new_insts.append(inst)
            bb.instructions = new_insts
    return n


def _make_inmaps(inputs, t_steps=T, ncores=NCORES):
    X = np.asarray(inputs["X"], np.float32)
    v_lengths = np.asarray(inputs["v_lengths"]).astype(np.int64)
    shared = _prep_shared(
        np.asarray(inputs["kernel_w"], np.float32), np.asarray(inputs["kernel_b"], np.float32),
        np.asarray(inputs["rec_w"], np.float32), np.asarray(inputs["rec_b"], np.float32),
        np.asarray(inputs["scale_w"], np.float32), np.asarray(inputs["scale_b"], np.float32),
        np.asarray(inputs["rescale_w"], np.float32), np.asarray(inputs["rescale_b"], np.float32),
        np.asarray(inputs["conv_w"], np.float32), np.asarray(inputs["conv_b"], np.float32),
        np.asarray(inputs["out_w"], np.float32), np.asarray(inputs["out_b"], np.float32))
    in_maps = []
    for c in range(ncores):
        bs = slice(c * BL, (c + 1) * BL)
        vl = v_lengths[bs]
        gidx = np.zeros((128, 5), np.int32)
        for p in range(128):
            for j in range(5):
                b = p % 64
                k = 2 * j + p // 64
                tb = int(vl[b]) - 1
                gidx[p, j] = (tb + k) * BL + b
        m = dict(shared)
        m["x"] = np.ascontiguousarray(X[bs, :t_steps, :])
        m["gidx"] = gidx
        in_maps.append(m)
    return in_maps


_NC_CACHE = {}


def kernel(**inputs) -> np.ndarray:
    t_steps = T
    if t_steps not in _NC_CACHE:
        _NC_CACHE[t_steps] = build_nc(t_steps)
    nc = _NC_CACHE[t_steps]
    in_maps = _make_inmaps(inputs, t_steps)
    res = run_bass_kernel_spmd(nc, in_maps, list(range(NCORES)))
    out = np.concatenate([res.results[c]["cur_out"] for c in range(NCORES)], axis=0)
    return out.astype(np.float32)

